# revision 42
# baseline (speedup 1.0000x reference)
# Mistral sliding-window attention (B=1, S=2048, H=4096, 32 q heads / 8 kv
# heads, window 4096 -> plain causal at this S) on 8 Trainium2 NeuronCores.
#
# Sharding: tensor-parallel over heads. Core c owns q heads 4c..4c+3 and kv
# head c. hidden_states is replicated (transposed on host to [H, S] so the
# contraction dim is the partition dim). Each core computes its attention
# output slice attn.T [512, S]; per-head AllGathers assemble the full
# [4096, S] while later heads still compute, and each core accumulates a
# 512-column slice of o_proj head-by-head; the host concatenates the 8
# column slices into the full output.
#
# All tensors are bf16 on the wire and in SBUF (fp32 accumulation in PSUM):
# bf16 halves HBM traffic AND enables Fast Weight Load on the PE (fp32
# stationaries load at 2 cycles/col with no FWL, which made LDWEIGHTS
# co-critical with the matmuls in the fp32r version of this kernel).
# Scores are computed transposed (S.T[kv, q]) so the P@V contraction needs
# no transposes of the probability tiles; softmax denominators come from an
# all-ones stationary matmul accumulated alongside P@V, and the causal mask
# is a host-precomputed staircase slice multiplied in after exp. Attention
# runs a rolling 2-deep software pipeline per (head, q-chunk): scores+exp
# for kv-pair p issue ahead of the PV/sum matmuls of pair p-2, so the PE
# never stalls on the ACT engine. o_proj for head h-2 interleaves with
# attention of head h to keep the PE dense through the collective tail.

from collections import deque
from contextlib import ExitStack

import ml_dtypes
import numpy as np

import concourse.bacc as bacc
import concourse.bass as bass
import concourse.mybir as mybir
import concourse.tile as tile
from concourse.bass_utils import run_bass_kernel_spmd


HIDDEN = 4096
NH = 32
NKV = 8
HD = 128
THETA = 10000.0
S = 2048
NCORES = 8

QH = NH // NCORES          # 4 q heads per core
DQ = QH * HD               # 512 (per-core q/attn width)
DOUT = DQ + 2 * HD         # 768 = q heads + k + v projection width
MT = DOUT // 128           # 6 projection m-tiles (0..3 q, 4 k, 5 v)
KT = HIDDEN // 128         # 32 contraction tiles
KG = 8                     # x-load group: k-tiles per DMA (1MB bf16)
TCH = 512                  # token chunk (matmul moving dim)
NTCH = S // TCH            # 4
KVT = S // 128             # 16 kv tiles
SCALE = 1.0 / float(np.sqrt(HD))

F32 = mybir.dt.float32
BF16 = mybir.dt.bfloat16
EXP = mybir.ActivationFunctionType.Exp


def _rope(nc, pool, src, dst, cs, sn):
    """dst = src*cos + rotate_half(src)*sin, in [d, tok] layout.

    src is [128, n] (PSUM f32); dst is [128, n] bf16; cs/sn are [64, n]
    (the two 64-row halves share frequencies). rotate_half: rows 0:64 get
    -src[64:128], rows 64:128 get src[0:64]. NOTE: a tensor_tensor op may
    not take two SBUF inputs at different base partitions (walrus
    NCC_IBIR297), so the cross-half products go through [64]-row tiles at
    base partition 0 with the PSUM operand carrying the partition offset.
    """
    top, bot = src[0:64, :], src[64:128, :]
    ta = pool.tile([64, TCH], BF16, name="rope_a")
    tb = pool.tile([64, TCH], BF16, name="rope_b")
    nc.vector.tensor_mul(ta, top, cs)
    nc.vector.tensor_mul(tb, bot, sn)
    nc.vector.tensor_sub(dst[0:64, :], ta, tb)
    nc.vector.tensor_mul(ta, bot, cs)
    nc.vector.tensor_mul(tb, top, sn)
    nc.vector.tensor_add(dst[64:128, :], ta, tb)


def build_kernel_body(ctx: ExitStack, tc: tile.TileContext, outs, ins):
    nc = tc.nc
    xT, wqkv, ow, cos_t, sin_t, stair = (
        ins["xT"], ins["wqkv"], ins["ow"], ins["cos_t"], ins["sin_t"], ins["stair"],
    )
    out = outs["out"]

    # per-head bounce + gather buffers so each head's AllGather can fire as
    # soon as that head's attention is done (overlaps comm with compute).
    # Head 3 (the last one computed) gathers per q-chunk instead, so its
    # final AllGather only covers 512 tokens and o_proj isn't gated on a
    # full-head collective at the tail.
    attn_loc = [nc.dram_tensor(f"attn_loc{h}", [HD, S], BF16).ap()
                for h in range(QH - 1)]
    attn_gat = [nc.dram_tensor(f"attn_gat{h}", [NCORES * HD, S], BF16,
                               addr_space="Shared").ap()
                for h in range(QH - 1)]
    attn_loc3 = [nc.dram_tensor(f"attn_loc3_{v}", [HD, 2 * TCH], BF16).ap()
                 for v in range(NTCH // 2)]
    attn_gat3 = [nc.dram_tensor(f"attn_gat3_{v}", [NCORES * HD, 2 * TCH],
                                BF16, addr_space="Shared").ap()
                 for v in range(NTCH // 2)]

    singles = ctx.enter_context(tc.tile_pool(name="singles", bufs=1))
    stair_sb = singles.tile([128, 896], BF16)
    ones_sb = singles.tile([128, 128], BF16)

    # persistent projection outputs, [d, tok] layout
    qT = singles.tile([128, QH, S], BF16)    # q head h -> qT[:, h, :]
    kT = singles.tile([128, S], BF16)
    V = singles.tile([128, KVT, HD], BF16)   # V[:, j, :] = [tok 128, d 128]

    # ---- phase 1: QKV projection + RoPE --------------------------------
    with (
        tc.tile_pool(name="wq", bufs=1) as wp,
        tc.tile_pool(name="xt", bufs=2) as xp,
        tc.tile_pool(name="rope", bufs=2) as rp,
        tc.tile_pool(name="p1ps", bufs=1, space="PSUM") as pp1,
        tc.tile_pool(name="ipsc", bufs=1, space="PSUM") as ip_sc,
        tc.tile_pool(name="ippa", bufs=1, space="PSUM") as ip_pa,
        tc.tile_pool(name="ippt", bufs=4) as ip_pt,
        tc.tile_pool(name="ipao", bufs=2) as ip_ao,
    ):
        cos_sb = wp.tile([64, S], F32)
        sin_sb = wp.tile([64, S], F32)
        vT = wp.tile([128, S], BF16)
        nc.vector.memset(ones_sb, 1.0)

        x3 = xT.rearrange("(k p) s -> p k s", p=128)
        wq4 = wqkv.rearrange("(g k p) d -> p g k d", p=128, k=KG)
        # DMA order: the first two k-tiles of weights+x land first so the
        # first matmul starts ~2us in; after that, weight groups interleave
        # just-in-time with chunk-0 x groups so x loads aren't queued behind
        # the whole 6.3MB weight stream.
        w00 = wp.tile([128, 1, DOUT], BF16, name="w00", tag="w00")
        w01 = wp.tile([128, KG - 1, DOUT], BF16, name="w01", tag="w01")
        w_sb = [None] + [wp.tile([128, KG, DOUT], BF16, name=f"w{g}",
                                 tag=f"w{g}")
                         for g in range(1, KT // KG)]
        x00 = xp.tile([128, 1, TCH], BF16, name="x00", tag="x00")
        x01 = xp.tile([128, KG - 1, TCH], BF16, name="x01", tag="x01")
        # x rides the ACT HWDGE ring, weights the Sync ring: the two first
        # loads land in parallel instead of queuing on one FIFO
        nc.sync.dma_start(out=w00, in_=wq4[:, 0, 0:1, :])
        nc.scalar.dma_start(out=x00, in_=x3[:, 0:1, 0:TCH])
        nc.sync.dma_start(out=w01, in_=wq4[:, 0, 1:KG, :])
        nc.scalar.dma_start(out=x01, in_=x3[:, 1:KG, 0:TCH])

        def wslice(kg, ki, m):
            if kg == 0:
                wt, i = (w00, ki) if ki < 1 else (w01, ki - 1)
            else:
                wt, i = w_sb[kg], ki
            return wt[:, i, m * 128:(m + 1) * 128]

        xg_t0 = [None] * (KT // KG)
        for g in range(1, KT // KG):
            nc.sync.dma_start(out=w_sb[g], in_=wq4[:, g, :, :])
            xg = xp.tile([128, KG, TCH], BF16, name="xg", tag=f"xg{g}")
            nc.scalar.dma_start(
                out=xg, in_=x3[:, g * KG:(g + 1) * KG, 0:TCH])
            xg_t0[g] = xg
        nc.sync.dma_start(out=cos_sb, in_=cos_t)
        nc.sync.dma_start(out=sin_sb, in_=sin_t)
        nc.sync.dma_start(out=stair_sb, in_=stair)

        def chunk_epilogue_v(t):
            # V = vT.T for this chunk's kv tiles via X-bar DMA transpose
            # (keeps the PE free and PSUM bank 7 unused by phase 1)
            for j in range(4 * t, 4 * t + 4):
                nc.sync.dma_start(out=V[:, j, :],
                                  in_=vT[:, j * 128:(j + 1) * 128],
                                  transpose=True)

        # Each chunk runs as three 2-projection passes over the k loop, so
        # phase-1 PSUM is 4 banks (2 live accumulators, double-buffered) and
        # each pass's RoPE overlaps the next pass's matmuls. That leaves 4
        # PSUM banks to interleave head-0's attention chunks 0-2 INTO phase
        # 1 at pass boundaries: head 0 finishes with phase 1, so its
        # AllGather (and the whole collective chain) starts ~30us earlier.
        GRP = [(QH, 0), (1, 2), (3, QH + 1)]  # k+q0 first, v last
        ist = {}
        ivq = deque()

        def iv_scores(c, p):
            st = ist.setdefault(c, dict(
                jmax=4 * c + 3, pts={},
                po=ip_pa.tile([128, TCH], F32, name="ipo", tag="ipo"),
                ps=ip_pa.tile([128, TCH], F32, name="ips", tag="ips")))
            sc = ip_sc.tile([128, 2, TCH], F32, name="isc", tag="isc")
            pt = ip_pt.tile([128, 2, TCH], BF16, name="ipt", tag="ipt")
            ss = [max(0, (2 * p + i - 4 * c) * 128) for i in range(2)]
            for i in range(2):
                j = 2 * p + i
                nc.tensor.matmul(
                    sc[:, i, ss[i]:], lhsT=kT[:, j * 128:(j + 1) * 128],
                    rhs=qT[:, 0, c * TCH + ss[i]:(c + 1) * TCH],
                    start=True, stop=True)
            if ss[1] == 0:
                nc.scalar.activation(pt, sc, EXP, scale=SCALE)
            else:
                for i in range(2):
                    nc.scalar.activation(pt[:, i, ss[i]:], sc[:, i, ss[i]:],
                                         EXP, scale=SCALE)
            for i in range(2):
                if 2 * p + i - 4 * c >= 0:
                    nc.vector.tensor_mul(
                        pt[:, i, ss[i]:], pt[:, i, ss[i]:],
                        stair_sb[:, 384:384 + TCH - ss[i]])
            st["pts"][p] = pt

        def iv_pv(c, p):
            st = ist[c]
            pt = st["pts"].pop(p)
            for i in range(2):
                j = 2 * p + i
                s = max(0, (j - 4 * c) * 128)
                nc.tensor.matmul(st["po"][:, s:], lhsT=V[:, j, :],
                                 rhs=pt[:, i, s:],
                                 start=(j == 0), stop=(j == st["jmax"]))
                nc.tensor.matmul(st["ps"][:, s:], lhsT=ones_sb,
                                 rhs=pt[:, i, s:],
                                 start=(j == 0), stop=(j == st["jmax"]))
            if not st["pts"] and p == st["jmax"] // 2:
                rec = ip_ao.tile([128, TCH], F32, name="irec")
                nc.vector.reciprocal(rec, st["ps"])
                ao = ip_ao.tile([128, TCH], BF16, name="iao")
                nc.vector.tensor_mul(ao, st["po"], rec)
                nc.sync.dma_start(
                    out=attn_loc[0][:, c * TCH:(c + 1) * TCH], in_=ao)
                del ist[c]

        # pairs of (h0, chunk c) placed at (t, group) boundaries where their
        # kv chunks (<= c) and q chunk (roped in group 0 of chunk c) exist
        TICKS = {(0, 2): [(0, 0)], (1, 0): [(0, 1)], (1, 1): [(1, 0)],
                 (1, 2): [(1, 1)], (2, 0): [(1, 2)], (2, 1): [(1, 3)],
                 (2, 2): [(2, 0)], (3, 0): [(2, 1), (2, 2)],
                 (3, 1): [(2, 3), (2, 4)], (3, 2): [(2, 5)]}

        for t in range(NTCH):
            if t == 0:
                xgs = xg_t0
            else:
                xgs = []
                for g in range(KT // KG):
                    xg = xp.tile([128, KG, TCH], BF16, name="xg",
                                 tag=f"xg{g}")
                    nc.scalar.dma_start(
                        out=xg,
                        in_=x3[:, g * KG:(g + 1) * KG, t * TCH:(t + 1) * TCH])
                    xgs.append(xg)
            cs = cos_sb[:, t * TCH:(t + 1) * TCH]
            sn = sin_sb[:, t * TCH:(t + 1) * TCH]
            for gi, ms in enumerate(GRP):
                ps2 = pp1.tile([128, 2, TCH], F32, name=f"pg{gi % 2}",
                               tag=f"pg{gi % 2}")
                for k in range(KT):
                    kg, ki = k // KG, k % KG
                    if t == 0 and kg == 0:
                        xs = x00[:, ki, :] if ki < 1 else x01[:, ki - 1, :]
                    else:
                        xs = xgs[kg][:, ki, :]
                    for mi, m in enumerate(ms):
                        nc.tensor.matmul(
                            ps2[:, mi, :], lhsT=wslice(kg, ki, m), rhs=xs,
                            start=(k == 0), stop=(k == KT - 1),
                        )
                # interleaved head-0 attention: PVs of the previous tick's
                # pairs, then this tick's scores (exp spans a full pass)
                while ivq:
                    iv_pv(*ivq.popleft())
                for cp in TICKS.get((t, gi), ()):
                    iv_scores(*cp)
                    ivq.append(cp)
                for mi, m in enumerate(ms):
                    if m == QH:
                        _rope(nc, rp, ps2[:, mi, :],
                              kT[:, t * TCH:(t + 1) * TCH], cs, sn)
                    elif m < QH:
                        _rope(nc, rp, ps2[:, mi, :],
                              qT[:, m, t * TCH:(t + 1) * TCH], cs, sn)
                    else:
                        nc.scalar.copy(out=vT[:, t * TCH:(t + 1) * TCH],
                                       in_=ps2[:, mi, :])
            chunk_epilogue_v(t)
        while ivq:
            iv_pv(*ivq.popleft())

    # ---- phase 2: attention heads with per-head AllGather ---------------
    ow3 = ow.rearrange("(k p) d -> p k d", p=128)
    out3 = out.rearrange("(b p) d -> p b d", p=128)
    ag3 = [attn_gat[h].rearrange("(r p) s -> p r s", p=128)
           for h in range(QH - 1)]
    ag3h = [attn_gat3[v].rearrange("(r p) s -> p r s", p=128)
            for v in range(NTCH // 2)]
    owp = ctx.enter_context(tc.tile_pool(name="ow", bufs=32))

    with (
        tc.tile_pool(name="pt", bufs=4) as ptp,
        tc.tile_pool(name="ao", bufs=2) as aop,
        tc.tile_pool(name="p2sc", bufs=2, space="PSUM") as pp2,
        tc.tile_pool(name="p2acc", bufs=2, space="PSUM") as pa2,
    ):
        # One continuous rolling pipeline across ALL (head, q-chunk) units:
        # the PV/sum matmuls trail the score+exp emission by DEPTH kv-pairs
        # globally, so the PE never drains at chunk or head boundaries (which
        # previously cost ~2us each plus a HAM re-throttle to 1.2GHz).
        def allgather(ins_ap, outs_ap):
            nc.gpsimd.collective_compute(
                "AllGather",
                mybir.AluOpType.bypass,
                ins=[ins_ap],
                outs=[outs_ap],
                replica_groups=[list(range(NCORES))],
            )

        ows = [[None] * NCORES for _ in range(QH)]
        state = {}
        pending = deque()
        DEPTH = 2

        def emit_scores(u, p):
            # Columns below rdiag*128 of a diagonal tile are fully masked:
            # skip them in the score matmul, exp, stair-mul, and PV (the PE
            # cost is the moving-dim length, so this trims the staircase).
            h, c = u
            st = state[u]
            sc = pp2.tile([128, 2, TCH], F32, name="sc", tag="sc")
            pt = ptp.tile([128, 2, TCH], BF16, name="pt", tag="pt")
            ss = [max(0, (2 * p + i - 4 * c) * 128) for i in range(2)]
            for i in range(2):
                j = 2 * p + i
                nc.tensor.matmul(
                    sc[:, i, ss[i]:], lhsT=kT[:, j * 128:(j + 1) * 128],
                    rhs=st["q"][:, ss[i]:], start=True, stop=True)
            if ss[1] == 0:
                nc.scalar.activation(pt, sc, EXP, scale=SCALE)
            else:
                for i in range(2):
                    nc.scalar.activation(pt[:, i, ss[i]:], sc[:, i, ss[i]:],
                                         EXP, scale=SCALE)
            for i in range(2):
                j = 2 * p + i
                if j - 4 * c >= 0:  # tile touches the causal diagonal
                    nc.vector.tensor_mul(
                        pt[:, i, ss[i]:], pt[:, i, ss[i]:],
                        stair_sb[:, 384:384 + TCH - ss[i]])
            st["pts"][p] = pt

        def epilogue(u):
            h, c = u
            st = state.pop(u)
            rec = aop.tile([128, TCH], F32, name="rec")
            nc.vector.reciprocal(rec, st["ps"])
            ao = aop.tile([128, TCH], BF16, name="ao")
            nc.vector.tensor_mul(ao, st["po"], rec)
            if h == QH - 1:
                half, idx = c // 2, c % 2
                nc.sync.dma_start(
                    out=attn_loc3[half][:, idx * TCH:(idx + 1) * TCH], in_=ao)
                if idx == 1:
                    allgather(attn_loc3[half][:, :], attn_gat3[half][:, :])
            else:
                nc.sync.dma_start(
                    out=attn_loc[h][:, c * TCH:(c + 1) * TCH], in_=ao)
                if c == NTCH - 1:
                    allgather(attn_loc[h][:, :], attn_gat[h][:, :])
                    if h == 0:
                        # o_proj weights DMA (4.2MB bf16) rides under attention
                        for hh in range(QH):
                            for r in range(NCORES):
                                owk = owp.tile([128, DQ], BF16, name="owk",
                                               tag="owk")
                                nc.sync.dma_start(
                                    out=owk, in_=ow3[:, r * QH + hh, :])
                                ows[hh][r] = owk

        def emit_pv(u, p):
            st = state[u]
            pt = st["pts"].pop(p)
            jmax = st["jmax"]
            h, c = u
            for i in range(2):
                j = 2 * p + i
                s = max(0, (j - 4 * c) * 128)
                nc.tensor.matmul(st["po"][:, s:], lhsT=V[:, j, :],
                                 rhs=pt[:, i, s:],
                                 start=(j == 0), stop=(j == jmax))
                nc.tensor.matmul(st["ps"][:, s:], lhsT=ones_sb,
                                 rhs=pt[:, i, s:],
                                 start=(j == 0), stop=(j == jmax))
            st["done"] += 1
            if st["done"] == st["npair"]:
                epilogue(u)

        # head 0's chunks 0-2 were computed inside phase 1; its AllGather
        # fires right after the (0,3) unit here
        units = [(0, NTCH - 1)] + [(h, c) for h in range(1, QH)
                                   for c in range(NTCH)]
        for u in units:
            h, c = u
            jmax = 4 * c + 3
            state[u] = dict(
                q=qT[:, h, c * TCH:(c + 1) * TCH], jmax=jmax,
                npair=(jmax + 1) // 2, done=0, pts={},
                po=pa2.tile([128, TCH], F32, name="po", tag="po"),
                ps=pa2.tile([128, TCH], F32, name="ps", tag="ps"))
            for p in range(state[u]["npair"]):
                emit_scores(u, p)
                pending.append((u, p))
                if len(pending) > DEPTH:
                    emit_pv(*pending.popleft())
        while pending:
            emit_pv(*pending.popleft())

    # ---- phase 3: o_proj, accumulated over all (head, rank) in PSUM.
    # Heads 0-2 of chunk g ("A block") accumulate as soon as the PE gets
    # there; head 3's quarter ("B block") is emitted one A block later so
    # the in-order PE stream never head-of-line blocks on head 3's
    # still-in-flight AllGather. B's at-load rides the ACT HWDGE ring so
    # its gather-wait can't block A-block loads on the Sync ring.
    with (
        tc.tile_pool(name="at", bufs=12) as atp,
        tc.tile_pool(name="stg", bufs=2) as stp,
        tc.tile_pool(name="p3ps", bufs=2, space="PSUM") as pp3,
    ):
        pcs = {}

        def oproj_a(g):
            pc = pp3.tile([128, 4, TCH], F32, name="pc", tag="pc")
            pcs[g] = pc
            for h in range(QH - 1):
                at = atp.tile([128, NCORES, TCH], BF16, name="at", tag="at")
                nc.sync.dma_start(
                    out=at, in_=ag3[h][:, :, g * TCH:(g + 1) * TCH])
                for r in range(NCORES):
                    for mi in range(4):
                        nc.tensor.matmul(
                            pc[:, mi, :],
                            lhsT=at[:, r, mi * 128:(mi + 1) * 128],
                            rhs=ows[h][r],
                            start=(h == 0 and r == 0), stop=False,
                        )

        def oproj_b(g):
            pc = pcs.pop(g)
            at = atp.tile([128, NCORES, TCH], BF16, name="at", tag="at")
            half, idx = g // 2, g % 2
            nc.scalar.dma_start(
                out=at, in_=ag3h[half][:, :, idx * TCH:(idx + 1) * TCH])
            h = QH - 1
            for r in range(NCORES):
                for mi in range(4):
                    nc.tensor.matmul(
                        pc[:, mi, :],
                        lhsT=at[:, r, mi * 128:(mi + 1) * 128],
                        rhs=ows[h][r],
                        start=False, stop=(r == NCORES - 1),
                    )
            stg = stp.tile([128, 4, TCH], F32, name="stg")
            nc.scalar.copy(out=stg, in_=pc)
            nc.sync.dma_start(out=out3[:, 4 * g:4 * g + 4, :], in_=stg)

        oproj_a(0)
        oproj_a(1)
        oproj_b(0)
        oproj_a(2)
        oproj_b(1)
        oproj_a(3)
        oproj_b(2)
        oproj_b(3)


_NC_CACHE = None


def build_program():
    global _NC_CACHE
    if _NC_CACHE is not None:
        return _NC_CACHE
    nc = bacc.Bacc("TRN2", target_bir_lowering=False, debug=False,
                   num_devices=NCORES)
    ins = {
        "xT": nc.dram_tensor("xT", [HIDDEN, S], BF16, kind="ExternalInput").ap(),
        "wqkv": nc.dram_tensor("wqkv", [HIDDEN, DOUT], BF16,
                               kind="ExternalInput").ap(),
        "ow": nc.dram_tensor("ow", [HIDDEN, DQ], BF16, kind="ExternalInput").ap(),
        "cos_t": nc.dram_tensor("cos_t", [64, S], F32, kind="ExternalInput").ap(),
        "sin_t": nc.dram_tensor("sin_t", [64, S], F32, kind="ExternalInput").ap(),
        "stair": nc.dram_tensor("stair", [128, 896], BF16,
                                kind="ExternalInput").ap(),
    }
    outs = {"out": nc.dram_tensor("out", [S, DQ], F32, kind="ExternalOutput").ap()}
    with tile.TileContext(nc) as tc:
        with ExitStack() as ctx:
            build_kernel_body(ctx, tc, outs, ins)
    nc.compile()
    _NC_CACHE = nc
    return nc


def make_in_maps(hidden_states, position_ids, q_w, k_w, v_w, o_w):
    bf16 = ml_dtypes.bfloat16
    x = np.asarray(hidden_states, dtype=np.float32).reshape(S, HIDDEN)
    xT = np.ascontiguousarray(x.T).astype(bf16)
    pos = np.asarray(position_ids).reshape(S).astype(np.float64)
    inv = 1.0 / (THETA ** (np.arange(0, HD, 2, dtype=np.float64) / HD))
    fr = inv[:, None] * pos[None, :]                       # [64, S]
    cos_t = np.cos(fr).astype(np.float32)
    sin_t = np.sin(fr).astype(np.float32)
    u = np.arange(896, dtype=np.int64)[None, :]
    kvi = np.arange(128, dtype=np.int64)[:, None]
    stair = ((u - kvi) >= 384).astype(bf16)                # [128, 896]

    q_w = np.asarray(q_w, dtype=np.float32)
    k_w = np.asarray(k_w, dtype=np.float32)
    v_w = np.asarray(v_w, dtype=np.float32)
    o_w = np.asarray(o_w, dtype=np.float32)

    in_maps = []
    for c in range(NCORES):
        wqkv = np.ascontiguousarray(np.concatenate(
            [q_w[:, c * DQ:(c + 1) * DQ],
             k_w[:, c * HD:(c + 1) * HD],
             v_w[:, c * HD:(c + 1) * HD]], axis=1)).astype(bf16)
        owc = np.ascontiguousarray(o_w[:, c * DQ:(c + 1) * DQ]).astype(bf16)
        in_maps.append({"xT": xT, "wqkv": wqkv, "ow": owc,
                        "cos_t": cos_t, "sin_t": sin_t, "stair": stair})
    return in_maps


def run(inputs: dict, trace: bool = False):
    """Run on the 8 NeuronCores; returns (full_output, BassKernelResults)."""
    nc = build_program()
    in_maps = make_in_maps(**inputs)
    res = run_bass_kernel_spmd(nc, in_maps, core_ids=list(range(NCORES)),
                               trace=trace)
    full = np.concatenate([res.results[c]["out"] for c in range(NCORES)], axis=1)
    return full.reshape(1, S, HIDDEN), res


def kernel(**inputs) -> np.ndarray:
    out, _ = run(inputs)
    return out
